# revision 1
# baseline (speedup 1.0000x reference)
"""Trainium2 Bass kernel for nn_MinMaxQuantizer (per-channel symmetric log_2 quantizer).

Math (per row c of x[C, D], half = 2**(n_bits-1)):
    rmax    = max(|x[c, :]|)
    max_val = floor(log2(rmax) + 0.5)                 # round-half-up of log2
    z       = max_val - (half - 1)                    # min kept exponent
    e       = round(log2(|x|))                        # per element
    out     = sign(x) * 2^e   if e >= z else 0

Implemented with exact bit tricks (no transcendentals).  Rounding log2 to the
nearest integer == "round the exponent up iff mantissa_bits >= 0x3504F4" (the
sqrt(2) boundary, which is irrational, so ties cannot occur).  All arithmetic
keeps intermediates either < 2^24 or multiples of 2^23, where the fp32-internal
ALUs are exact.  Bitwise and arith ops are never mixed in one instruction
(BIR verifier rule).

Per-element chain (x bits viewed as uint32), engine in brackets:
    m = x & 0x7FFFFF                       [DVE TS bitwise]        uint32
    g = Sign(m - 3474675.5)                [ACT Sign]              int8 {-1,+1}
    h = Identity(g*2^-(z+1) + 3*2^-(z+1))  [ACT Identity, AP scale/bias] bf16 (exact)
    p = x & 0xFF800000                     [DVE TS bitwise]        uint32 = bits(sign*2^floor)
    q = int16(p.f32 * h)                   [DVE TT mult]           int16;
        |v|<1 converts to 0, kept values are exact ints in [-128, 128]
    out = Copy(q * 2^z)                    [ACT Copy, scale AP]    f32

Sharding: rows 4096 -> 8 cores x 512 rows, zero communication.
"""

import sys

import numpy as np

_REPO = "/opt/trn_rl_repo"

N_ROWS = 4096
N_COLS = 11008
N_CORES = 8
ROWS_PER_CORE = N_ROWS // N_CORES  # 512
P = 128
N_SLAB = ROWS_PER_CORE // P  # 4
N_CH = 4
W = N_COLS // N_CH  # 2752

_MANT_MASK = 0x007FFFFF
_SIGN_BIAS = -3474675.5  # Sign(m + this) > 0 iff mantissa >= 0x3504F4
_CARRY_K = float(0x800000 - 0x3504F4)  # 4913932.0
_CARRY_BIT = 0x00800000
_EXP_MASK = 0x7F800000
_SIGNEXP_MASK = 0xFF800000
_INV_CONST = float(254 << 23)  # bits(2^-z)     = this - bits(2^z)
_INVH_CONST = float(253 << 23)  # bits(2^-(z+1)) = this - bits(2^z)


def _ensure_path():
    if _REPO not in sys.path:
        sys.path.insert(0, _REPO)


def _build(n_bits: int):
    _ensure_path()
    import concourse.bacc as bacc
    import concourse.mybir as mybir
    import concourse.tile as tile

    dt = mybir.dt
    Alu = mybir.AluOpType
    Act = mybir.ActivationFunctionType
    X = mybir.AxisListType.X

    half_sub = float((2 ** (n_bits - 1) - 1) << 23)  # bits offset: max_val -> z

    nc = bacc.Bacc("TRN2", target_bir_lowering=False, debug=False, num_devices=N_CORES)
    x_ext = nc.dram_tensor("x", [ROWS_PER_CORE, N_COLS], dt.float32, kind="ExternalInput")
    out_ext = nc.dram_tensor("out", [ROWS_PER_CORE, N_COLS], dt.float32, kind="ExternalOutput")

    with tile.TileContext(nc) as tc:
        with (
            tc.tile_pool(name="const", bufs=1) as cpool,
            tc.tile_pool(name="xp", bufs=7) as xp,
            tc.tile_pool(name="stat", bufs=2) as stat,
            tc.tile_pool(name="mp", bufs=2) as mp,
            tc.tile_pool(name="gp", bufs=3) as gp,
            tc.tile_pool(name="hp", bufs=3) as hp,
            tc.tile_pool(name="pp", bufs=2) as pp,
            tc.tile_pool(name="qp", bufs=3) as qp,
            tc.tile_pool(name="op", bufs=3) as op,
        ):
            sign_bias = cpool.tile([P, 1], dt.float32, name="sign_bias")
            nc.vector.memset(sign_bias[:], _SIGN_BIAS)

            def load_and_reduce(s):
                r0 = s * P
                xts = []
                rpart = stat.tile([P, N_CH], dt.float32, tag="rpart", name=f"rpart{s}")
                for j in range(N_CH):
                    c0 = j * W
                    xt = xp.tile([P, W], dt.float32, tag="x", name=f"x{s}_{j}")
                    nc.sync.dma_start(out=xt[:], in_=x_ext[r0 : r0 + P, c0 : c0 + W])
                    nc.vector.tensor_reduce(
                        out=rpart[:, j : j + 1],
                        in_=xt[:],
                        axis=X,
                        op=Alu.max,
                        apply_absolute_value=True,
                    )
                    xts.append(xt)
                return xts, rpart

            staged = load_and_reduce(0)
            for s in range(N_SLAB):
                r0 = s * P
                xts, rpart = staged
                if s + 1 < N_SLAB:
                    staged = load_and_reduce(s + 1)

                rmax = stat.tile([P, 1], dt.float32, tag="rmax", name=f"rmax{s}")
                nc.vector.tensor_reduce(out=rmax[:], in_=rpart[:], axis=X, op=Alu.max)

                # --- per-row params: bits(2^z), bits(2^-(z+1)) -----------------
                rb = rmax[:].bitcast(dt.uint32)
                mm = stat.tile([P, 1], dt.uint32, tag="mm", name=f"mm{s}")
                nc.vector.tensor_scalar(
                    out=mm[:], in0=rb, scalar1=_MANT_MASK, scalar2=None,
                    op0=Alu.bitwise_and,
                )
                t1 = stat.tile([P, 1], dt.uint32, tag="t1", name=f"t1_{s}")
                nc.vector.tensor_scalar(
                    out=t1[:], in0=mm[:], scalar1=_CARRY_K, scalar2=None, op0=Alu.add,
                )
                t2 = stat.tile([P, 1], dt.uint32, tag="t2", name=f"t2_{s}")
                nc.vector.tensor_scalar(
                    out=t2[:], in0=t1[:], scalar1=_CARRY_BIT, scalar2=None,
                    op0=Alu.bitwise_and,
                )
                pe = stat.tile([P, 1], dt.uint32, tag="pe", name=f"pe{s}")
                nc.vector.tensor_scalar(
                    out=pe[:], in0=rb, scalar1=_EXP_MASK, scalar2=None,
                    op0=Alu.bitwise_and,
                )
                zb = stat.tile([P, 1], dt.uint32, tag="zb", name=f"zb{s}")
                nc.vector.tensor_tensor(out=zb[:], in0=pe[:], in1=t2[:], op=Alu.add)
                zbits = stat.tile([P, 1], dt.uint32, tag="zbits", name=f"zbits{s}")
                nc.vector.tensor_scalar(
                    out=zbits[:], in0=zb[:], scalar1=half_sub, scalar2=None,
                    op0=Alu.subtract,
                )
                ihb = stat.tile([P, 1], dt.uint32, tag="ihb", name=f"ihb{s}")
                nc.vector.tensor_scalar(
                    out=ihb[:], in0=zbits[:], scalar1=-1.0, scalar2=_INVH_CONST,
                    op0=Alu.mult, op1=Alu.add,
                )
                z_f = zbits[:].bitcast(dt.float32)
                ihz_f = ihb[:].bitcast(dt.float32)
                ihz3 = stat.tile([P, 1], dt.float32, tag="ihz3", name=f"ihz3_{s}")
                nc.vector.tensor_scalar(
                    out=ihz3[:], in0=ihz_f, scalar1=3.0, scalar2=None, op0=Alu.mult,
                )

                # --- per-element quantize chain -------------------------------
                for j in range(N_CH):
                    c0 = j * W
                    xbv = xts[j][:].bitcast(dt.uint32)
                    mt = mp.tile([P, W], dt.uint32, tag="m", name=f"m{s}_{j}")
                    nc.vector.tensor_scalar(
                        out=mt[:], in0=xbv, scalar1=_MANT_MASK, scalar2=None,
                        op0=Alu.bitwise_and,
                    )
                    gt = gp.tile([P, W], dt.int8, tag="g", name=f"g{s}_{j}")
                    nc.scalar.activation(
                        out=gt[:], in_=mt[:], func=Act.Sign, bias=sign_bias[:], scale=1.0,
                    )
                    ht = hp.tile([P, W], dt.bfloat16, tag="h", name=f"h{s}_{j}")
                    nc.scalar.activation(
                        out=ht[:], in_=gt[:], func=Act.Identity, bias=ihz3[:], scale=ihz_f,
                    )
                    pt = pp.tile([P, W], dt.uint32, tag="p", name=f"p{s}_{j}")
                    nc.vector.tensor_scalar(
                        out=pt[:], in0=xbv, scalar1=_SIGNEXP_MASK, scalar2=None,
                        op0=Alu.bitwise_and,
                    )
                    qt = qp.tile([P, W], dt.int16, tag="q", name=f"q{s}_{j}")
                    nc.vector.tensor_tensor(
                        out=qt[:], in0=pt[:].bitcast(dt.float32), in1=ht[:],
                        op=Alu.mult,
                    )
                    ot = op.tile([P, W], dt.float32, tag="o", name=f"o{s}_{j}")
                    nc.scalar.activation(
                        out=ot[:], in_=qt[:], func=Act.Copy, bias=0.0, scale=z_f,
                    )
                    nc.sync.dma_start(out=out_ext[r0 : r0 + P, c0 : c0 + W], in_=ot[:])

    nc.compile()
    return nc


def kernel(x, n_bits):
    _ensure_path()
    from concourse.bass_utils import run_bass_kernel_spmd

    x = np.ascontiguousarray(np.asarray(x, dtype=np.float32))
    assert x.shape == (N_ROWS, N_COLS), x.shape
    nb = int(np.asarray(n_bits))

    nc = _build(nb)
    in_maps = [
        {"x": x[i * ROWS_PER_CORE : (i + 1) * ROWS_PER_CORE]} for i in range(N_CORES)
    ]
    res = run_bass_kernel_spmd(nc, in_maps, list(range(N_CORES)))
    return np.concatenate([res.results[i]["out"] for i in range(N_CORES)], axis=0)

